# revision 27
# baseline (speedup 1.0000x reference)
"""DCMMSR sparse attention TRN2 kernel.

Sharding: 16 (batch, head) pairs -> 8 cores, 2 adjacent heads of one batch
per core (head-parallel). Out-projection is computed per-core as a partial
sum over its 2 heads' feature rows; host sums the 4 partials per batch and
adds bo (the unshard step).

Math notes (B=2, S=512, E=512, H=8, d=64, WSZ=64, TOPK=4, W=8):
 - S % WSZ == 0 so the reference's padding mask is all-true and wts = 1/64.
 - The coarse softmax cancels against the log-bias inside the fine softmax
   up to a per-query constant:
     probs = softmax_n( q.k_n * scale + s_{w(n)}/t + (0 if w sel else -inf) )
   where s_w = sum_{s in w} (q.k_s)^2 / (64 t |q|^2 |k_s|^2). So no gather
   and no explicit coarse softmax - only the top-4 selection mask, applied
   densely over all 512 keys with non-selected windows biased to -200
   (exp -> exactly 0, while keeping the selected windows' s_w bias exact).
 - q is pre-scaled by d^-0.5 at projection time; scores are invariant
   because qinv2 is computed from the scaled q (exact cancellation), and
   the fine logits then need no further scaling.
 - Window routing (top-4 selection) is extremely sensitive to score
   perturbations: any fp16 rounding on the q/k/score path flips enough
   borderline windows to push rel-err to ~1.5-2.4e-2 (gate is 2e-2). So
   projections consume fp16 x/W (DMA halved) but qf/kf and the whole score
   pipeline stay fp32; only the v/attn-weight/out paths run in fp16.
 - Per-head normalization (1/Z) is applied token-major after the per-head
   out-projection partials, so Z's reciprocal runs on [128, 4] tiles
   instead of [1, 512] (which costs ~2us on DVE).
"""

import numpy as np

import concourse.bass as bass
import concourse.mybir as mybir
import concourse.tile as tile
from concourse import bacc
from concourse.bass import ts
from concourse.bass_utils import run_bass_kernel_spmd

F32 = mybir.dt.float32
F16 = mybir.dt.float16

B, S, E = 2, 512, 512
H, D, WSZ, TOPK = 8, 64, 64, 4
NW = S // WSZ          # 8 windows
NC = 8                 # cores
HPC = 2                # heads per core
D2 = HPC * D           # 128 feature rows per core
P = 128
NCHUNK = S // P        # 4
SCALE = D ** -0.5      # 0.125
SENT = 200.0           # selection sentinel; exp(x-200) flushes to 0


def build_kernel(inv64t: float, fast: bool):
    """Per-core Tile program. inv64t = 1/(64*t) is baked in.

    fast=True : all-fp16 matmul path (score path sees fp16-rounded inputs;
                fails the 2e-2 gate on this data - kept for experiments).
    fast=False: q/k/dots/score path in fp32; v/attn path fp16.
    """
    nc = bacc.Bacc(
        "TRN2",
        target_bir_lowering=False,
        debug=False,
        enable_asserts=False,
        num_devices=NC,
    )

    DT_QK = F16 if fast else F32   # qf/kf storage -> dots operand dtype
    DT_SC = F16 if fast else F32   # dsq / bselk / ksq / qsq dtype

    dt_in = {}
    for name, shape, dtt in [
        ("xq", [P, NCHUNK * S], F16), ("xk", [P, NCHUNK * S], F16),
        ("xv", [P, NCHUNK * S], F16),
        ("wall", [P, 3 * NCHUNK * P], F16),       # wq|wk|wv chunks, 3KB lines
        ("wo2", [D, 2 * E], F16),                 # wo head0 | head1, 2KB lines
        ("bias3", [D2, 4], F32),                  # bq*SCALE, bk, bv, ones
        ("blob", [P, P + D + NCHUNK * NW], F16),  # eye | ones | bsel
        ("sel8", [NW, NCHUNK * P], F16),
    ]:
        dt_in[name] = nc.dram_tensor(name, shape, dtt, kind="ExternalInput").ap()
    out_dram = nc.dram_tensor("out", [S, E], F16, kind="ExternalOutput").ap()

    with tile.TileContext(nc) as tc, nc.allow_low_precision(reason="fp16 path"):
        with (
            tc.tile_pool(name="const", bufs=1) as cpool,
            tc.tile_pool(name="sbig", bufs=4) as sbig,
            tc.tile_pool(name="spt", bufs=6) as spt,
            tc.tile_pool(name="sdsq", bufs=2) as sdsq,
            tc.tile_pool(name="ssm", bufs=4) as ssm,
            tc.tile_pool(name="stiny", bufs=8) as stiny,
            tc.tile_pool(name="pdots", bufs=4, space="PSUM") as pdots,
            tc.tile_pool(name="pbig", bufs=2, space="PSUM") as pbig,
            tc.tile_pool(name="psm", bufs=2, space="PSUM") as psm,
        ):
            # ---- loads: 2KB+ per-partition lines, compute-priority order ----
            def load(name, shape, src):
                t = cpool.tile(shape, src.dtype, tag=name)
                nc.sync.dma_start(out=t[:], in_=src)
                return t

            bias3 = load("bias3", [D2, 4], dt_in["bias3"])
            blob = load("blob", [P, P + D + NCHUNK * NW], dt_in["blob"])
            sel8 = load("sel8", [NW, NCHUNK * P], dt_in["sel8"])
            eye = blob[:, 0:P]
            ones = blob[:, P:P + D]
            bsel = blob[:, P + D:]

            wall = load("wall", [P, 3 * NCHUNK * P], dt_in["wall"])

            def wslice(i, c):
                return wall[:, (i * NCHUNK + c) * P:(i * NCHUNK + c + 1) * P]

            def load_x(xname):
                halves = []
                for hh in range(2):
                    halves.append(load(
                        f"{xname}{hh}", [P, 2 * S],
                        dt_in[xname][:, hh * 2 * S:(hh + 1) * 2 * S],
                    ))
                return halves

            xqt = load_x("xq")
            xkt = load_x("xk")
            xvt = load_x("xv")
            wo2 = load("wo2", [D, 2 * E], dt_in["wo2"])

            # ---- projections (feature-major [o2, token]) ----
            def proj(wi, x_t, bcol, dt_out, scale):
                ps = pbig.tile([P, S], F32, tag="pb")
                for c in range(NCHUNK):
                    nc.tensor.matmul(
                        ps[:], wslice(wi, c), x_t[c // 2][:, (c % 2) * S:(c % 2 + 1) * S],
                        start=(c == 0), stop=(c == NCHUNK - 1),
                    )
                sb = cpool.tile([P, S], dt_out, tag=f"f{bcol}")
                nc.vector.tensor_scalar(
                    sb[:], ps[:], scale, bias3[:, bcol:bcol + 1],
                    op0=mybir.AluOpType.mult, op1=mybir.AluOpType.add,
                )
                return sb

            qf = proj(0, xqt, 0, DT_QK, SCALE)   # [128 o2, 512 q] pre-scaled
            kf = proj(1, xkt, 1, DT_QK, 1.0)     # [128 o2, 512 s]
            vf = proj(2, xvt, 2, F16, 1.0)

            # squared features (for norms) on gpsimd
            qsq = cpool.tile([P, S], DT_SC, tag="qsq")
            nc.gpsimd.tensor_mul(qsq[:], qf[:], qf[:])
            ksq = cpool.tile([P, S], DT_SC, tag="ksq")
            nc.gpsimd.tensor_mul(ksq[:], kf[:], kf[:])

            kn_ps = psm.tile([P, HPC * NCHUNK], F32, tag="ps")
            qn_ps = psm.tile([P, HPC * NCHUNK], F32, tag="ps")
            kinv2 = stiny.tile([P, HPC * NCHUNK], F32, tag="kinv2")
            qinv2 = stiny.tile([P, HPC * NCHUNK], F32, tag="qinv2")
            v_tok = [None] * NCHUNK
            num_sb = [None, None]
            zqi = [None, None]

            def emit_norms():
                for h in range(HPC):
                    hs = slice(h * D, (h + 1) * D)
                    onecol = ones[hs, 0:1] if fast else bias3[hs, 3:4]
                    for c in range(NCHUNK):
                        col = h * NCHUNK + c
                        nc.tensor.matmul(
                            kn_ps[:, col:col + 1], ksq[hs, ts(c, P)], onecol,
                            skip_group_check=True,
                        )
                        nc.tensor.matmul(
                            qn_ps[:, col:col + 1], qsq[hs, ts(c, P)], onecol,
                            skip_group_check=True,
                        )
                nc.vector.reciprocal(kinv2[:], kn_ps[:])
                nc.vector.reciprocal(qinv2[:], qn_ps[:])

            def emit_vtok():
                # v token-major, [128 s', 130]: 0:64 h0-v, 64 ones,
                # 65:129 h1-v, 129 ones
                for c in range(NCHUNK):
                    vt_ps = pbig.tile([P, 2 * S], F16, tag="pb")
                    nc.tensor.transpose(vt_ps[:, 0:P], vf[:, ts(c, P)], eye)
                    vt = cpool.tile([P, 2 * (D + 1)], F16, tag=f"vtok{c}")
                    if c < 2:
                        nc.vector.tensor_copy(vt[:, 0:D], vt_ps[:, 0:D])
                        nc.vector.tensor_copy(
                            vt[:, D + 1:2 * D + 1], vt_ps[:, D:2 * D])
                    else:
                        nc.scalar.copy(vt[:, 0:D], vt_ps[:, 0:D])
                        nc.scalar.copy(
                            vt[:, D + 1:2 * D + 1], vt_ps[:, D:2 * D])
                    nc.vector.tensor_scalar(
                        vt[:, D:D + 1], vt_ps[:, 0:1], 0.0, 1.0,
                        op0=mybir.AluOpType.mult, op1=mybir.AluOpType.add,
                    )
                    nc.vector.tensor_scalar(
                        vt[:, 2 * D + 1:2 * D + 2], vt_ps[:, 0:1], 0.0, 1.0,
                        op0=mybir.AluOpType.mult, op1=mybir.AluOpType.add,
                    )
                    v_tok[c] = vt

            for h in range(HPC):
                hs = slice(h * D, (h + 1) * D)

                # dots^T chunks: [128 s, 512 q] = k_chunk^T q (4 kept live)
                dots = []
                for c in range(NCHUNK):
                    dc = pdots.tile([P, S], F32, tag="dots")
                    nc.tensor.matmul(dc[:], kf[hs, ts(c, P)], qf[hs, :])
                    dots.append(dc)

                if h == 0:
                    emit_norms()
                    emit_vtok()

                # dsq via scalar square (single PSUM read, Scalar engine)
                dsqs = []
                for c in range(NCHUNK):
                    dsq = sdsq.tile([P, S], DT_SC, tag=f"dsq{c}")
                    nc.scalar.activation(
                        dsq[:], dots[c][:],
                        mybir.ActivationFunctionType.Square,
                    )
                    dsqs.append(dsq)

                # window-selector columns scaled by kinv2/(64 t)
                bselk = ssm.tile([P, NCHUNK * NW], DT_SC, tag="bselk")
                for c in range(NCHUNK):
                    nc.vector.tensor_scalar(
                        bselk[:, ts(c, NW)], bsel[:, ts(c, NW)],
                        kinv2[:, h * NCHUNK + c:h * NCHUNK + c + 1],
                        inv64t,
                        op0=mybir.AluOpType.mult, op1=mybir.AluOpType.mult,
                    )

                # routing scores q-major; top-4 mask; bias.
                # V does psum-read + max8; GpSimd does mask+bias (SBUF only).
                bias_q = ssm.tile([P, NCHUNK * NW], F16, tag="biasq")
                for qc in range(NCHUNK):
                    scq_ps = psm.tile([P, NW], F32, tag="ps")
                    for c in range(NCHUNK):
                        nc.tensor.matmul(
                            scq_ps[:],
                            dsqs[c][:, ts(qc, P)],
                            bselk[:, ts(c, NW)],
                            start=(c == 0), stop=(c == NCHUNK - 1),
                        )
                    scores_q = stiny.tile([P, NW], F32, tag="scq")
                    nc.vector.tensor_scalar(
                        scores_q[:], scq_ps[:],
                        qinv2[:, h * NCHUNK + qc:h * NCHUNK + qc + 1], None,
                        op0=mybir.AluOpType.mult,
                    )
                    srt = stiny.tile([P, 8], F32, tag="srt")
                    nc.vector.max(srt[:], scores_q[:])
                    # m2 = SENT where below threshold, else 0; bias = s - m2
                    m2 = stiny.tile([P, NW], F32, tag="m")
                    nc.gpsimd.tensor_scalar(
                        m2[:], scores_q[:],
                        srt[:, TOPK - 1:TOPK], SENT,
                        op0=mybir.AluOpType.is_lt, op1=mybir.AluOpType.mult,
                    )
                    nc.gpsimd.tensor_sub(
                        bias_q[:, ts(qc, NW)], scores_q[:], m2[:],
                    )

                # transpose bias back to w-major [8, 512]
                bw_ps = psm.tile([NW, S], F16, tag="ps")
                for c in range(NCHUNK):
                    nc.tensor.transpose(
                        bw_ps[:, ts(c, P)], bias_q[:, ts(c, NW)], eye,
                    )
                bias_w = ssm.tile([NW, S], F16, tag="biasw")
                nc.vector.tensor_copy(bias_w[:], bw_ps[:])

                # accumulate expanded bias into dots psum, then exp and PV
                num_ps = pbig.tile([D + 1, S], F32, tag="pb")
                for c in range(NCHUNK):
                    nc.tensor.matmul(
                        dots[c][:], sel8[:, ts(c, P)], bias_w[:],
                        start=False, stop=True, skip_group_check=True,
                    )
                    pt = spt.tile([P, S], F16, tag="pt")
                    nc.scalar.activation(
                        pt[:], dots[c][:], mybir.ActivationFunctionType.Exp,
                    )
                    nc.tensor.matmul(
                        num_ps[:], v_tok[c][:, h * (D + 1):(h + 1) * (D + 1)],
                        pt[:],
                        start=(c == 0), stop=(c == NCHUNK - 1),
                    )

                # Z q-major: copy row D to SBUF, transpose per chunk, recip
                z16 = stiny.tile([1, S], F32, tag="z16")
                nc.vector.tensor_copy(z16[:], num_ps[D:D + 1, :])
                zq_ps = psm.tile([P, NCHUNK], F32, tag="ps")
                for c in range(NCHUNK):
                    nc.tensor.transpose(
                        zq_ps[:, c:c + 1], z16[0:1, ts(c, P)], bias3[0:1, 3:4],
                    )
                zq = stiny.tile([P, NCHUNK], F32, tag=f"zqi{h}")
                nc.vector.reciprocal(zq[:], zq_ps[:])
                zqi[h] = zq

                nsb = sbig.tile([D, S], F16, tag=f"numsb{h}")
                nc.vector.tensor_copy(nsb[:], num_ps[0:D, :])
                num_sb[h] = nsb

            # ---- per-head out-projection partials, then 1/Z combine.
            # op tiles live in the (now idle) pdots banks for pipeline depth.
            for c in range(NCHUNK):
                op0 = pdots.tile([P, E], F32, tag="dots")
                nc.tensor.matmul(op0[:], num_sb[0][:, ts(c, P)], wo2[:, 0:E])
                op1 = pdots.tile([P, E], F32, tag="dots")
                nc.tensor.matmul(op1[:], num_sb[1][:, ts(c, P)], wo2[:, E:2 * E])
                t0 = sbig.tile([P, E], F32, tag="t0")
                nc.vector.tensor_scalar(
                    t0[:], op0[:], zqi[0][:, c:c + 1], None,
                    op0=mybir.AluOpType.mult,
                )
                t1 = sbig.tile([P, E], F32, tag="t1")
                nc.scalar.activation(
                    t1[:], op1[:], mybir.ActivationFunctionType.Copy,
                    scale=zqi[1][:, c:c + 1],
                )
                ot = sbig.tile([P, E], F16, tag="osb")
                nc.vector.tensor_add(ot[:], t0[:], t1[:])
                nc.sync.dma_start(out=out_dram[ts(c, P), :], in_=ot[:])

    nc.compile()
    return nc


_CACHE = {}


def _consts():
    eye = np.eye(P, dtype=np.float16)
    ones = np.ones((P, D), dtype=np.float16)
    # sel8[w, c*128 + s'] = 1 if w == 2c + s'//64
    sel8 = np.zeros((NW, NCHUNK * P), dtype=np.float16)
    for c in range(NCHUNK):
        for sp in range(P):
            sel8[2 * c + sp // WSZ, c * P + sp] = 1.0
    # bsel[s', c*8 + w] = 1 if w == 2c + s'//64
    bsel = np.zeros((P, NCHUNK * NW), dtype=np.float16)
    for c in range(NCHUNK):
        for sp in range(P):
            bsel[sp, c * NW + 2 * c + sp // WSZ] = 1.0
    blob = np.concatenate([eye, ones, bsel], axis=1)
    return blob, sel8


def kernel(query, key, value, Wq, bq, Wk, bk, Wv, bv, Wo, bo, temp,
           _want_perf=False, _fast=False):
    query = np.asarray(query, dtype=np.float32)
    key = np.asarray(key, dtype=np.float32)
    value = np.asarray(value, dtype=np.float32)
    t = float(np.clip(np.asarray(temp, dtype=np.float32), 0.1, None)[0])
    inv64t = 1.0 / (WSZ * t)

    ck = (inv64t, _fast)
    if ck not in _CACHE:
        _CACHE[ck] = build_kernel(inv64t, _fast)
    nc = _CACHE[ck]

    blob, sel8 = _consts()
    WqT = np.asarray(Wq, dtype=np.float32).T.astype(np.float16)
    WkT = np.asarray(Wk, dtype=np.float32).T.astype(np.float16)
    WvT = np.asarray(Wv, dtype=np.float32).T.astype(np.float16)
    WoT = np.asarray(Wo, dtype=np.float32).T.astype(np.float16)
    bqs = np.asarray(bq, dtype=np.float32) * SCALE
    bk = np.asarray(bk, dtype=np.float32)
    bv = np.asarray(bv, dtype=np.float32)
    bo = np.asarray(bo, dtype=np.float32)
    x16 = {}
    for nm, arr in (("q", query), ("k", key), ("v", value)):
        for b in range(B):
            # pre-chunked layout: x16[p, c*S+t] = x[b][t, c*P+p] so every
            # partition line is one contiguous 4KB read
            xt = arr[b].T.astype(np.float16)            # [E, S]
            x16[nm, b] = np.ascontiguousarray(
                xt.reshape(NCHUNK, P, S).transpose(1, 0, 2).reshape(
                    P, NCHUNK * S))

    in_maps = []
    for core in range(NC):
        b = core // (NC // B)
        hp = core % (NC // B)
        cols = slice(hp * D2, (hp + 1) * D2)
        bias3 = np.stack([bqs[cols], bk[cols], bv[cols],
                          np.ones(D2, dtype=np.float32)], axis=1)
        # wall[p, (i*4+c)*128 + o] = W_i^T[c*128+p, cols.start+o]
        wall = np.empty((P, 3 * NCHUNK * P), dtype=np.float16)
        for i, WT in enumerate((WqT, WkT, WvT)):
            wc = WT[:, cols]                     # [512, 128]
            for c in range(NCHUNK):
                wall[:, (i * NCHUNK + c) * P:(i * NCHUNK + c + 1) * P] = \
                    wc[c * P:(c + 1) * P, :]
        woc = WoT[cols, :]                       # [128, 512]
        wo2 = np.concatenate([woc[0:D, :], woc[D:D2, :]], axis=1)  # [64,1024]
        in_maps.append({
            "xq": x16["q", b],
            "xk": x16["k", b],
            "xv": x16["v", b],
            "wall": np.ascontiguousarray(wall),
            "wo2": np.ascontiguousarray(wo2),
            "bias3": np.ascontiguousarray(bias3),
            "blob": blob, "sel8": sel8,
        })

    res = run_bass_kernel_spmd(nc, in_maps, core_ids=list(range(NC)),
                               trace=_want_perf)

    out = np.zeros((B, S, E), dtype=np.float32)
    for core in range(NC):
        b = core // (NC // B)
        out[b] += res.results[core]["out"].astype(np.float32)
    out += bo.reshape(1, 1, E)

    if _want_perf:
        return out, res
    return out
